# revision 46
# baseline (speedup 1.0000x reference)
"""MultiHeadSelfAttention2D Trainium2 kernel (8 NeuronCores).

Sharding: core i computes attention for (head i%4, batch i//4); an 8-way
AllToAll then redistributes attention outputs so core i finishes the final
1x1-conv + PReLU + LN + residual for time-slice [256*i, 256*i+256) of both
batches.

Per-core layouts:
  - x supplied as 4 channel-quarter packs xq[q] [128, T, 8]:
      row fi*16+ci -> x[b, q*16+ci, t, g*8+fi]  (free dims t, g)
  - QKV 1x1 convs: 4 accumulating PE matmuls (channel quarters) with
    f-block-diagonal weights; contraction K = 128 = 8 f-values x 16 ch.
    Psum row packing (all 32-aligned):
      Q/K: out [32, TT] at offset 32*(g%4), bank qb=g//4:
           row = 32*(g%4) + fj*4 + hc ; d  = qb*128 + row ; f = g*8+fj
      V:   out [128, TT], bank vb=g:  row = fj*16 + vc ; dv = vb*128 + row
  - PReLU = max(y, alpha*y) on VectorE (measured faster than the ScalarE
    AFT.Prelu path, which is kept behind prelu_act=True; AFT.Lrelu
    ignores its alpha argument - hardwired 0.01 slope).
  - build_kernel(reps=K) wraps phase A+attention and phase 2 in tc.For_i
    hardware loops (identical outputs each iteration) for differential
    on-device timing; collectives must stay outside hardware loops (the
    runtime pre-registers a static collective sequence - looping one
    desyncs the mesh).
  - Channel-LN: PE stats-matmul (group means of z, z^2) + PE
    broadcast-matmul returning rstd / mu*rstd to data rows.
  - Attention: S^T[k,q] = Kf @ Qf^T (fp32r); exp on ScalarE (no max sub);
    P^T bf16; row sums via ones-matmul; O^T = Vf.T @ P^T, Vf bf16 [t, dv].
  - Phase 2 f-pairing: f = p*32 + fh; contraction K = 128 = 2f x 64ch;
    out rows p*64+o. Output [128, 32, B, 256] packed; host unshuffles.
"""
import sys
sys.path.insert(0, "/opt/trn_rl_repo")
sys.path.insert(0, "/opt/trn_rl_repo/concourse")

import contextlib

import numpy as np
import ml_dtypes

import concourse.bass as bass
import concourse.mybir as mybir
import concourse.tile as tile
from concourse import bacc
from concourse.bass_utils import run_bass_kernel_spmd
from concourse.masks import make_identity

F32 = mybir.dt.float32
F32R = mybir.dt.float32r
BF16 = mybir.dt.bfloat16
AFT = mybir.ActivationFunctionType
ALU = mybir.AluOpType

B, C, T, F = 2, 64, 2048, 64
H, HC, VC = 4, 4, 16
D = HC * F
DV = VC * F
NCORES = 8
TT = 512
NTT = T // TT
KCH = T // 128
TS = T // NCORES
SCALE = 1.0 / float(np.sqrt(D))
EPS = 1e-5


def r32(ap):
    return ap


def build_kernel(replica_groups, no_collective=False, cfg=None, phases=(1, 1, 1),
                 reps=1, loop_stagger=False, force_loop=False,
                 prelu_act=False):
    cfg = {**{'xsp': 1, 'zw': 4, 'zw2': 2, 'chw': 1, 'pb': 4, 'tr': 2, 'st': 2,
              'sps': 2, 'osb': 3, 'p2w': 3, 'pps': 2, 'st2': 2, 'b2': 2},
           **(cfg or {})}
    nc = bacc.Bacc("TRN2", target_bir_lowering=False, debug=False,
                   num_devices=NCORES)

    xq_in = [nc.dram_tensor(f"xq{q}", [128, T, 8], BF16,
                            kind="ExternalInput").ap() for q in range(4)]
    xres2 = nc.dram_tensor("xres2", [128, 32, B, TS], F32,
                           kind="ExternalInput").ap()
    wq4 = nc.dram_tensor("wq4", [4, 128, 32], BF16, kind="ExternalInput").ap()
    wk4 = nc.dram_tensor("wk4", [4, 128, 32], BF16, kind="ExternalInput").ap()
    wv4 = nc.dram_tensor("wv4", [4, 128, 128], BF16, kind="ExternalInput").ap()
    wp2 = nc.dram_tensor("wp2", [128, 128], BF16, kind="ExternalInput").ap()
    qb_col = nc.dram_tensor("qb_col", [128, 1], F32, kind="ExternalInput").ap()
    qa_col = nc.dram_tensor("qa_col", [128, 1], F32, kind="ExternalInput").ap()
    kb_col = nc.dram_tensor("kb_col", [128, 1], F32, kind="ExternalInput").ap()
    ka_col = nc.dram_tensor("ka_col", [128, 1], F32, kind="ExternalInput").ap()
    vb_col = nc.dram_tensor("vb_col", [128, 1], F32, kind="ExternalInput").ap()
    va_col = nc.dram_tensor("va_col", [128, 1], F32, kind="ExternalInput").ap()
    pb_col = nc.dram_tensor("pb_col", [128, 1], F32, kind="ExternalInput").ap()
    pa_col = nc.dram_tensor("pa_col", [128, 1], F32, kind="ExternalInput").ap()
    g32p4 = nc.dram_tensor("g32p4", [4, 128, 128], F32,
                           kind="ExternalInput").ap()
    g8p4 = nc.dram_tensor("g8p4", [4, 128, 128], F32,
                          kind="ExternalInput").ap()
    g2p4 = nc.dram_tensor("g2p4", [4, 128, 128], F32,
                          kind="ExternalInput").ap()
    bqk_sel = nc.dram_tensor("bqk_sel", [4, 128, 128], F32,
                             kind="ExternalInput").ap()
    bv_sel = nc.dram_tensor("bv_sel", [4, 128, 128], F32,
                            kind="ExternalInput").ap()
    b2_sel = nc.dram_tensor("b2_sel", [4, 128, 128], F32,
                            kind="ExternalInput").ap()
    outp = nc.dram_tensor("outp", [128, 32, B, TS], F32,
                          kind="ExternalOutput").ap()

    def _rep(tag):
        return (tc.For_i(0, reps, name=tag, staggered_reset=loop_stagger)
                if (reps > 1 or force_loop) else contextlib.nullcontext())

    with tile.TileContext(nc) as tc:
        with tc.tile_pool(name="persist", bufs=1) as persist, \
             tc.tile_pool(name="dram", bufs=1, space="DRAM") as dram:
            eps_col = persist.tile([128, 1], F32)
            wp2_sb = persist.tile([128, 128], BF16)
            pb_sb = persist.tile([128, 1], F32)
            pa_sb = persist.tile([128, 1], F32)
            g2p4_f = persist.tile([128, 4, 128], F32)
            g2p4_sb = persist.tile([128, 4, 128], F32R)
            b2_sel_f = persist.tile([128, 4, 128], F32)
            b2_sel_sb = persist.tile([128, 4, 128], F32R)

            send = dram.tile([NCORES, DV, TS], BF16)
            recv = dram.tile([NCORES, DV, TS], BF16)

            with _rep("repA"), tc.tile_pool(name="qkvp", bufs=1) as qkvp:
                nc.vector.memset(eps_col[:], EPS)
                nc.sync.dma_start(wp2_sb[:], wp2[:])
                nc.sync.dma_start(pb_sb[:], pb_col[:])
                nc.sync.dma_start(pa_sb[:], pa_col[:])
                for j in range(4):
                    nc.sync.dma_start(g2p4_f[:, j, :], g2p4[j])
                    nc.scalar.activation(out=g2p4_sb[:, j, :],
                                         in_=g2p4_f[:, j, :], func=AFT.Copy)
                    nc.sync.dma_start(b2_sel_f[:, j, :], b2_sel[j])
                    nc.scalar.activation(out=b2_sel_sb[:, j, :],
                                         in_=b2_sel_f[:, j, :], func=AFT.Copy)
                zq = qkvp.tile([128, 2, T], F32R)
                zk = qkvp.tile([128, 2, T], F32R)
                vf = qkvp.tile([128, KCH, DV], BF16)
                rinv = qkvp.tile([1, T], F32R)
                rinvb = qkvp.tile([128, T], F32)
                ones_f = qkvp.tile([1, 128], F32)
                nc.vector.memset(ones_f[:], 1.0)
                ones_row = qkvp.tile([1, 128], F32R)
                nc.scalar.activation(out=ones_row[:], in_=ones_f[:],
                                     func=AFT.Copy)
                ones_bf = qkvp.tile([128, 1], BF16)
                nc.vector.memset(ones_bf[:], 1.0)
                ident_bf = qkvp.tile([128, 128], BF16)
                make_identity(nc, ident_bf[:])
                wq4_sb = qkvp.tile([128, 4, 32], BF16)
                wk4_sb = qkvp.tile([128, 4, 32], BF16)
                wv4_sb = qkvp.tile([128, 4, 128], BF16)
                for q in range(4):
                    nc.sync.dma_start(wq4_sb[:, q, :], wq4[q])
                    nc.sync.dma_start(wk4_sb[:, q, :], wk4[q])
                    nc.sync.dma_start(wv4_sb[:, q, :], wv4[q])
                cols = {}
                for nm, src in (("qb", qb_col), ("qa", qa_col),
                                ("kb", kb_col), ("ka", ka_col),
                                ("vb", vb_col), ("va", va_col)):
                    t_ = qkvp.tile([128, 1], F32, name=f"{nm}_sb")
                    nc.sync.dma_start(t_[:], src[:])
                    cols[nm] = t_
                g32p4_f = qkvp.tile([128, 4, 128], F32)
                g8p4_f = qkvp.tile([128, 4, 128], F32)
                g32p4_sb = qkvp.tile([128, 4, 128], F32R)
                g8p4_sb = qkvp.tile([128, 4, 128], F32R)
                for j in range(4):
                    nc.sync.dma_start(g32p4_f[:, j, :], g32p4[j])
                    nc.sync.dma_start(g8p4_f[:, j, :], g8p4[j])
                    nc.scalar.activation(out=g32p4_sb[:, j, :],
                                         in_=g32p4_f[:, j, :], func=AFT.Copy)
                    nc.scalar.activation(out=g8p4_sb[:, j, :],
                                         in_=g8p4_f[:, j, :], func=AFT.Copy)
                bqk_sel_f = qkvp.tile([128, 4, 128], F32)
                bv_sel_f = qkvp.tile([128, 4, 128], F32)
                bqk_sel_sb = qkvp.tile([128, 4, 128], F32R)
                bv_sel_sb = qkvp.tile([128, 4, 128], F32R)
                for j in range(4):
                    nc.sync.dma_start(bqk_sel_f[:, j, :], bqk_sel[j])
                    nc.sync.dma_start(bv_sel_f[:, j, :], bv_sel[j])
                    nc.scalar.activation(out=bqk_sel_sb[:, j, :],
                                         in_=bqk_sel_f[:, j, :], func=AFT.Copy)
                    nc.scalar.activation(out=bv_sel_sb[:, j, :],
                                         in_=bv_sel_f[:, j, :], func=AFT.Copy)

                # ---------------- Phase A ----------------
                with tc.tile_pool(name="xsp", bufs=cfg["xsp"]) as xsp, \
                     tc.tile_pool(name="zw", bufs=cfg["zw"]) as zw, \
                     tc.tile_pool(name="ztp", bufs=12) as ztp, \
                     tc.tile_pool(name="zw2", bufs=cfg["zw2"]) as zw2, \
                     tc.tile_pool(name="chw", bufs=cfg["chw"]) as chw, \
                     tc.tile_pool(name="pb_ps", bufs=cfg["pb"], space="PSUM") as pb_ps, \
                     tc.tile_pool(name="tr_ps", bufs=cfg["tr"], space="PSUM") as tr_ps, \
                     tc.tile_pool(name="st_ps", bufs=cfg["st"], space="PSUM") as st_ps:

                    def prelu_drain(ps_t, bc, ac, tag):
                        zt = ztp.tile([128, TT], F32R, tag="zt", name=tag)
                        if prelu_act:
                            nc.scalar.activation(out=zt[:], in_=ps_t[:],
                                                 func=AFT.Prelu,
                                                 bias=bc[:], scale=1.0,
                                                 alpha=ac[:])
                            z2t = zw2.tile([128, TT], F32R, tag="z2t")
                            nc.scalar.activation(out=z2t[:], in_=zt[:],
                                                 func=AFT.Square)
                            return zt, z2t
                        nc.scalar.activation(out=zt[:], in_=ps_t[:],
                                             func=AFT.Identity,
                                             bias=bc[:], scale=1.0)
                        za = zw2.tile([128, TT], F32, tag="za")
                        nc.vector.tensor_scalar_mul(out=za[:], in0=zt[:],
                                                    scalar1=ac[:])
                        nc.vector.tensor_tensor(out=zt[:], in0=zt[:],
                                                in1=za[:], op=ALU.max)
                        z2t = zw2.tile([128, TT], F32R, tag="z2t")
                        nc.vector.tensor_tensor(out=z2t[:], in0=zt[:],
                                                in1=zt[:], op=ALU.mult)
                        return zt, z2t

                    def chain(mu_ps, m2_ps, tag):
                        # mu/m2 live in PSUM; DVE can read at most one PSUM
                        # operand per instruction, so square mu on ScalarE.
                        mus = mu_ps
                        var = chw.tile([128, TT], F32, tag=f"var{tag}")
                        nc.scalar.activation(out=var[:], in_=mus[:],
                                             func=AFT.Square)
                        nc.vector.tensor_tensor(out=var[:], in0=m2_ps[:],
                                                in1=var[:], op=ALU.subtract)
                        rstd = chw.tile([128, TT], F32R, tag=f"rstd{tag}")
                        nc.scalar.activation(out=rstd[:], in_=var[:],
                                             func=AFT.Sqrt,
                                             bias=eps_col[:], scale=1.0)
                        with nc.allow_low_precision(reason="fp32r rstd"):
                            nc.vector.reciprocal(out=rstd[:], in_=rstd[:])
                        nmr = chw.tile([128, TT], F32R, tag=f"nmr{tag}")
                        nc.vector.tensor_tensor(out=nmr[:], in0=mus[:],
                                                in1=rstd[:], op=ALU.mult)
                        return rstd, nmr

                    for tt in range(NTT if phases[0] else 0):
                        t0 = tt * TT
                        xq_sb = []
                        for q in range(4):
                            xt = xsp.tile([128, TT, 8], BF16, tag=f"xq{q}")
                            nc.sync.dma_start(xt[:], xq_in[q][:, t0:t0 + TT, :])
                            xq_sb.append(xt)

                        # ---- project + drain + stats for all 12 banks ----
                        # Group-stats land directly in [128,TT] PSUM tiles:
                        # each bank's matmul uses a position-shifted
                        # stationary (sel^T/groupsize, nonzero only in its
                        # 32 output rows) and the 4 banks ACCUMULATE into one
                        # tile. (Writing [32,TT] slices via tile_position at
                        # 32-alignment is illegal for fp32r - NCC_IXCG864 -
                        # and [32,TT] staging + ScalarE copies through a
                        # 2-buf pool was the dominant phase-A stall.)
                        qk_out = {}
                        v_out = {}
                        mu_qk = st_ps.tile([128, TT], F32, tag="st",
                                           name="mu_qk")
                        m2_qk = st_ps.tile([128, TT], F32, tag="st",
                                           name="m2_qk")
                        for kind, w4, bc, ac, soff in (
                                ("q", wq4_sb, cols["qb"], cols["qa"], 0),
                                ("k", wk4_sb, cols["kb"], cols["ka"], 64)):
                            banks = [pb_ps.tile([128, TT], F32, tag="pb",
                                                name=f"pb_{kind}{i}")
                                     for i in range(2)]
                            for q in range(4):
                                for g in range(8):
                                    off = 32 * (g % 4)
                                    nc.tensor.matmul(
                                        banks[g // 4][off:off + 32, :],
                                        r32(w4[:, q, :]),
                                        r32(xq_sb[q][:, :, g]),
                                        start=(q == 0), stop=(q == 3),
                                        tile_position=(0, off),
                                        skip_group_check=True)
                            for bk in range(2):
                                zt, z2t = prelu_drain(banks[bk], bc, ac,
                                                      f"zt_{kind}{bk}")
                                pos = soff // 32 + bk
                                nc.tensor.matmul(mu_qk[:],
                                                 r32(g32p4_sb[:, pos, :]),
                                                 zt[:],
                                                 start=(pos == 0),
                                                 stop=(pos == 3))
                                nc.tensor.matmul(m2_qk[:],
                                                 r32(g32p4_sb[:, pos, :]),
                                                 z2t[:],
                                                 start=(pos == 0),
                                                 stop=(pos == 3))
                                qk_out[(kind, bk)] = zt
                        rstd_qk, nmr_qk = chain(mu_qk, m2_qk, "qk")

                        rstd_v = [None, None]
                        nmr_v = [None, None]
                        for half in range(2):
                            mu_vh = st_ps.tile([128, TT], F32, tag="st",
                                               name=f"mu_v{half}")
                            m2_vh = st_ps.tile([128, TT], F32, tag="st",
                                               name=f"m2_v{half}")
                            for pair in range(2):
                                banks = [pb_ps.tile([128, TT], F32,
                                                    tag="pb",
                                                    name=f"pb_v{half}{pair}{i}")
                                         for i in range(2)]
                                for q in range(4):
                                    for g2 in range(2):
                                        g = half * 4 + pair * 2 + g2
                                        nc.tensor.matmul(
                                            banks[g2][:],
                                            r32(wv4_sb[:, q, :]),
                                            r32(xq_sb[q][:, :, g]),
                                            start=(q == 0), stop=(q == 3))
                                for g2 in range(2):
                                    bk = half * 4 + pair * 2 + g2
                                    zt, z2t = prelu_drain(
                                        banks[g2], cols["vb"], cols["va"],
                                        f"zt_v{bk}")
                                    j = pair * 2 + g2
                                    nc.tensor.matmul(mu_vh[:],
                                                     r32(g8p4_sb[:, j, :]),
                                                     zt[:],
                                                     start=(j == 0),
                                                     stop=(j == 3))
                                    nc.tensor.matmul(m2_vh[:],
                                                     r32(g8p4_sb[:, j, :]),
                                                     z2t[:],
                                                     start=(j == 0),
                                                     stop=(j == 3))
                                    v_out[bk] = zt
                            rstd_v[half], nmr_v[half] = chain(
                                mu_vh, m2_vh, f"v{half}")
                        rstd_v0, nmr_v0 = rstd_v[0], nmr_v[0]
                        rstd_v1, nmr_v1 = rstd_v[1], nmr_v[1]

                        # ---- all normalizations ----
                        for kind, bk in (("q", 0), ("q", 1), ("k", 0),
                                         ("k", 1)):
                            pos = (0 if kind == "q" else 2) + bk
                            zt = qk_out[(kind, bk)]
                            rb = pb_ps.tile([128, TT], F32, tag="pb")
                            nb = pb_ps.tile([128, TT], F32, tag="pb")
                            nc.tensor.matmul(rb[:],
                                             r32(bqk_sel_sb[:, pos, :]),
                                             r32(rstd_qk[:]),
                                             start=True, stop=True)
                            nc.tensor.matmul(nb[:],
                                             r32(bqk_sel_sb[:, pos, :]),
                                             r32(nmr_qk[:]),
                                             start=True, stop=True)
                            nc.vector.tensor_tensor(out=zt[:], in0=zt[:],
                                                    in1=rb[:], op=ALU.mult)
                            dstz = zq if kind == "q" else zk
                            nc.vector.tensor_tensor(
                                out=dstz[:, bk, t0:t0 + TT],
                                in0=zt[:], in1=nb[:], op=ALU.subtract)
                        for bk in range(8):
                            half = bk // 4
                            j = bk % 4
                            zt = v_out[bk]
                            rr = rstd_v0 if half == 0 else rstd_v1
                            nn_ = nmr_v0 if half == 0 else nmr_v1
                            rb = pb_ps.tile([128, TT], F32, tag="pb")
                            nb = pb_ps.tile([128, TT], F32, tag="pb")
                            nc.tensor.matmul(rb[:],
                                             r32(bv_sel_sb[:, j, :]),
                                             r32(rr[:]),
                                             start=True, stop=True)
                            nc.tensor.matmul(nb[:],
                                             r32(bv_sel_sb[:, j, :]),
                                             r32(nn_[:]),
                                             start=True, stop=True)
                            nc.vector.tensor_tensor(out=zt[:], in0=zt[:],
                                                    in1=rb[:],
                                                    op=ALU.mult)
                            zvn = zw.tile([128, TT], BF16, tag="zvn")
                            nc.vector.tensor_tensor(out=zvn[:], in0=zt[:],
                                                    in1=nb[:],
                                                    op=ALU.subtract)
                            for tch in range(TT // 128):
                                trp = tr_ps.tile([128, 128], BF16,
                                                 tag="trp")
                                nc.tensor.transpose(
                                    trp[:],
                                    zvn[:, tch * 128:(tch + 1) * 128],
                                    ident_bf[:])
                                nc.scalar.activation(
                                    out=vf[:, (t0 // 128) + tch,
                                           bk * 128:(bk + 1) * 128],
                                    in_=trp[:], func=AFT.Copy)

                # ---------------- Attention ----------------
                with tc.tile_pool(name="ptp", bufs=1) as ptp, \
                     tc.tile_pool(name="osb", bufs=cfg["osb"]) as osb, \
                     tc.tile_pool(name="sps", bufs=cfg["sps"], space="PSUM") as sps, \
                     tc.tile_pool(name="ops", bufs=1, space="PSUM") as ops, \
                     tc.tile_pool(name="rps", bufs=1, space="PSUM") as rps:
                    pT = ptp.tile([128, KCH, T], BF16)
                    for qt in range(NTT if phases[1] else 0):
                        q0 = qt * TT
                        for kc in range(KCH):
                            ps_s = sps.tile([128, TT], F32, tag="ps_s")
                            nc.tensor.matmul(
                                ps_s[:],
                                r32(zk[:, 0, kc * 128:(kc + 1) * 128]),
                                r32(zq[:, 0, q0:q0 + TT]),
                                start=True, stop=False)
                            nc.tensor.matmul(
                                ps_s[:],
                                r32(zk[:, 1, kc * 128:(kc + 1) * 128]),
                                r32(zq[:, 1, q0:q0 + TT]),
                                start=False, stop=True)
                            nc.scalar.activation(
                                out=pT[:, kc, q0:q0 + TT], in_=ps_s[:],
                                func=AFT.Exp, scale=SCALE)
                        ps_r = rps.tile([1, TT], F32, tag="ps_r")
                        for kc in range(KCH):
                            nc.tensor.matmul(
                                ps_r[:], ones_bf[:], pT[:, kc, q0:q0 + TT],
                                start=(kc == 0), stop=(kc == KCH - 1))
                        with nc.allow_low_precision(reason="fp32r rinv"):
                            nc.vector.reciprocal(out=rinv[:, q0:q0 + TT],
                                                 in_=ps_r[:])
                        rbb = rps.tile([128, TT], F32, tag="rbb")
                        nc.tensor.matmul(rbb[:], r32(ones_row[:]),
                                         r32(rinv[:, q0:q0 + TT]),
                                         start=True, stop=True)
                        nc.scalar.activation(out=rinvb[:, q0:q0 + TT],
                                             in_=rbb[:], func=AFT.Copy)

                    for dvc in range(DV // 128 if phases[1] else 0):
                        ps_o = [ops.tile([128, TT], F32, tag=f"ps_o{qt}",
                                         name=f"ps_o{qt}")
                                for qt in range(NTT)]
                        for kc in range(KCH):
                            for qt in range(NTT):
                                nc.tensor.matmul(
                                    ps_o[qt][:],
                                    vf[:, kc, dvc * 128:(dvc + 1) * 128],
                                    pT[:, kc, qt * TT:(qt + 1) * TT],
                                    start=(kc == 0), stop=(kc == KCH - 1))
                        for qt in range(NTT):
                            ot = osb.tile([128, TT], BF16, tag="ot")
                            nc.vector.tensor_tensor(
                                out=ot[:], in0=ps_o[qt][:],
                                in1=rinvb[:, qt * TT:(qt + 1) * TT],
                                op=ALU.mult)
                            for j in range(2):
                                nc.sync.dma_start(
                                    send[2 * qt + j,
                                         dvc * 128:(dvc + 1) * 128, :],
                                    ot[:, j * TS:(j + 1) * TS])

            if no_collective:
                # timing-only variant: model the exchange as a local copy
                nc.sync.dma_start(recv[:], send[:])
            else:
                nc.gpsimd.collective_compute(
                    "AllToAll", ALU.bypass,
                    replica_groups=replica_groups,
                    ins=[send[:].opt()], outs=[recv[:].opt()])

            # ---------------- Phase 2 ----------------
            with _rep("repB"), \
                 tc.tile_pool(name="zpp", bufs=1) as zpp, \
                 tc.tile_pool(name="p2w", bufs=cfg["p2w"]) as p2w, \
                 tc.tile_pool(name="p2c", bufs=2) as p2c, \
                 tc.tile_pool(name="pps", bufs=cfg["pps"], space="PSUM") as pps, \
                 tc.tile_pool(name="s2ps", bufs=cfg["st2"], space="PSUM") as s2ps, \
                 tc.tile_pool(name="b2ps", bufs=cfg["b2"], space="PSUM") as b2ps:
                zp_all = zpp.tile([128, 32, B * TS], F32R)
                o2a = zpp.tile([128, 32, B, TS], BF16)
                for p in range(2):
                    for bb in range(B):
                        for h in range(4):
                            for fhb in range(4):
                                base = (4 * p + fhb) * 128
                                nc.sync.dma_start(
                                    o2a[p * 64 + h * 16:p * 64 + h * 16 + 16,
                                        fhb * 8:(fhb + 1) * 8, bb, :],
                                    recv[bb * 4 + h, base:base + 128, :]
                                    .rearrange("(fhl v) t -> v fhl t", v=16)
                                    if False else
                                    recv[bb * 4 + h, base:base + 128, :]
                                    .rearrange("(fhl v) t -> v fhl t", fhl=8))
                for grp in range(8 if phases[2] else 0):
                    mu2 = s2ps.tile([128, B * TS], F32, tag="st2",
                                    name="mu2")
                    m22 = s2ps.tile([128, B * TS], F32, tag="st2",
                                    name="m22")
                    for j4 in range(4):
                        fh = grp * 4 + j4
                        ps_p = pps.tile([128, B * TS], F32, tag="ps_p")
                        nc.tensor.matmul(
                            ps_p[:], r32(wp2_sb[:]),
                            r32(o2a[:, fh, :, :].rearrange(
                                "r b t -> r (b t)")),
                            start=True, stop=True)
                        zpt = zp_all[:, fh, :]
                        if prelu_act:
                            nc.scalar.activation(out=zpt, in_=ps_p[:],
                                                 func=AFT.Prelu,
                                                 bias=pb_sb[:], scale=1.0,
                                                 alpha=pa_sb[:])
                            z2p = p2w.tile([128, B * TS], F32R, tag="z2p")
                            nc.scalar.activation(out=z2p[:], in_=zpt,
                                                 func=AFT.Square)
                        else:
                            yp = p2w.tile([128, B * TS], F32, tag="yp")
                            nc.scalar.activation(out=yp[:], in_=ps_p[:],
                                                 func=AFT.Identity,
                                                 bias=pb_sb[:], scale=1.0)
                            ya = p2w.tile([128, B * TS], F32, tag="ya")
                            nc.vector.tensor_scalar_mul(out=ya[:], in0=yp[:],
                                                        scalar1=pa_sb[:])
                            nc.vector.tensor_tensor(out=zpt, in0=yp[:],
                                                    in1=ya[:], op=ALU.max)
                            z2p = p2w.tile([128, B * TS], F32R, tag="z2p")
                            nc.vector.tensor_tensor(out=z2p[:], in0=zpt,
                                                    in1=zpt, op=ALU.mult)
                        nc.tensor.matmul(mu2[:], r32(g2p4_sb[:, j4, :]), zpt,
                                         start=(j4 == 0), stop=(j4 == 3))
                        nc.tensor.matmul(m22[:], r32(g2p4_sb[:, j4, :]),
                                         z2p[:],
                                         start=(j4 == 0), stop=(j4 == 3))

                    mus2 = mu2
                    var2 = p2c.tile([128, B * TS], F32, tag="var2")
                    nc.scalar.activation(out=var2[:], in_=mus2[:],
                                         func=AFT.Square)
                    nc.vector.tensor_tensor(out=var2[:], in0=m22[:],
                                            in1=var2[:], op=ALU.subtract)
                    rstd2 = p2c.tile([128, B * TS], F32R, tag="rstd2")
                    nc.scalar.activation(out=rstd2[:], in_=var2[:],
                                         func=AFT.Sqrt,
                                         bias=eps_col[:], scale=1.0)
                    with nc.allow_low_precision(reason="fp32r rstd2"):
                        nc.vector.reciprocal(out=rstd2[:], in_=rstd2[:])
                    nmr2 = p2c.tile([128, B * TS], F32R, tag="nmr2")
                    nc.vector.tensor_tensor(out=nmr2[:], in0=mus2[:],
                                            in1=rstd2[:], op=ALU.mult)

                    for j4 in range(4):
                        fh = grp * 4 + j4
                        rb2 = b2ps.tile([128, B * TS], F32, tag="rb2")
                        nb2 = b2ps.tile([128, B * TS], F32, tag="nb2")
                        nc.tensor.matmul(rb2[:], r32(b2_sel_sb[:, j4, :]),
                                         r32(rstd2[:]), start=True, stop=True)
                        nc.tensor.matmul(nb2[:], r32(b2_sel_sb[:, j4, :]),
                                         r32(nmr2[:]), start=True, stop=True)
                        t1 = p2w.tile([128, B * TS], F32, tag="t1")
                        nc.vector.tensor_tensor(out=t1[:],
                                                in0=zp_all[:, fh, :],
                                                in1=rb2[:], op=ALU.mult)
                        nc.vector.tensor_tensor(out=t1[:], in0=t1[:],
                                                in1=nb2[:], op=ALU.subtract)
                        xr = p2w.tile([128, B * TS], F32, tag="xr")
                        nc.sync.dma_start(
                            xr[:],
                            xres2[:, fh, :, :].rearrange("r b t -> r (b t)"))
                        nc.vector.tensor_tensor(out=t1[:], in0=t1[:],
                                                in1=xr[:], op=ALU.add)
                        nc.sync.dma_start(
                            outp[:, fh, :, :].rearrange("r b t -> r (b t)"),
                            t1[:])
    nc.compile()
    return nc


def make_inputs(x, Wq, bq, aq, Wk, bk, ak, Wv, bv, av, Wp, bp, ap_s):
    r = np.arange(128)

    def wquarters(w):  # [O, C] -> [4, 128, 8*O] f-block-diagonal quarters
        o = w.shape[0]
        m = np.zeros((4, 128, 8 * o), np.float32)
        for q in range(4):
            for fi in range(8):
                m[q, fi * 16:(fi + 1) * 16, fi * o:(fi + 1) * o] = \
                    w[:, q * 16:(q + 1) * 16].T
        return m

    def blockdiag2(w):  # [O, C] -> [128, 2*O]
        o = w.shape[0]
        m = np.zeros((128, 2 * o), np.float32)
        m[0:64, 0:o] = w.T
        m[64:128, o:2 * o] = w.T
        return m

    bqk_sel_np = np.zeros((4, 128, 128), np.float32)
    bv_sel_np = np.zeros((4, 128, 128), np.float32)
    b2_sel_np = np.zeros((4, 128, 128), np.float32)
    for pos in range(4):
        bqk_sel_np[pos, pos * 32 + r // 4, r] = 1.0
        bv_sel_np[pos, pos * 32 + r // 16, r] = 1.0
        b2_sel_np[pos, pos * 32 + r // 64, r] = 1.0
    # position-shifted stats stationaries: sel^T / group_size
    g32p4_np = np.ascontiguousarray(bqk_sel_np.transpose(0, 2, 1)) / 4
    g8p4_np = np.ascontiguousarray(bv_sel_np.transpose(0, 2, 1)) / 16
    g2p4_np = np.ascontiguousarray(b2_sel_np.transpose(0, 2, 1)) / 64

    def to_xq(xb):  # [C, T, F] -> [4][128, T, 8]
        out = []
        for q in range(4):
            blk = xb[q * 16:(q + 1) * 16]          # [16, T, 64]
            blk = blk.reshape(16, T, 8, 8)          # ci, t, g, fi
            blk = np.moveaxis(blk, (0, 1, 2, 3), (1, 2, 3, 0))  # fi,ci,t,g
            out.append(np.ascontiguousarray(
                blk.reshape(128, T, 8), np.float32))
        return out

    in_maps = []
    for i in range(NCORES):
        h, b = i % 4, i // 4
        xqs = to_xq(x[b])
        xres_s = x[:, :, i * TS:(i + 1) * TS, :]
        xr2 = np.empty((128, 32, B, TS), np.float32)
        xr2[0:64] = np.moveaxis(xres_s[:, :, :, 0:32], (0, 1, 2, 3),
                                (2, 0, 3, 1))
        xr2[64:128] = np.moveaxis(xres_s[:, :, :, 32:64], (0, 1, 2, 3),
                                  (2, 0, 3, 1))
        im = {
            "xres2": xr2,
            "wq4": wquarters(Wq[h]).astype(ml_dtypes.bfloat16),
            "wk4": wquarters(Wk[h]).astype(ml_dtypes.bfloat16),
            "wv4": wquarters(Wv[h]).astype(ml_dtypes.bfloat16),
            "wp2": blockdiag2(Wp).astype(ml_dtypes.bfloat16),
            "qb_col": np.tile(bq[h], 32).astype(np.float32)[:, None],
            "qa_col": np.full((128, 1), aq[h], np.float32),
            "kb_col": np.tile(bk[h], 32).astype(np.float32)[:, None],
            "ka_col": np.full((128, 1), ak[h], np.float32),
            "vb_col": np.tile(bv[h], 8).astype(np.float32)[:, None],
            "va_col": np.full((128, 1), av[h], np.float32),
            "pb_col": np.concatenate([bp, bp]).astype(np.float32)[:, None],
            "pa_col": np.full((128, 1), ap_s, np.float32),
            "g32p4": g32p4_np, "g8p4": g8p4_np, "g2p4": g2p4_np,
            "bqk_sel": bqk_sel_np, "bv_sel": bv_sel_np, "b2_sel": b2_sel_np,
        }
        for q in range(4):
            im[f"xq{q}"] = xqs[q].astype(ml_dtypes.bfloat16)
        in_maps.append(im)
    return in_maps


def assemble_output(results):
    out = np.empty((B, C, T, F), np.float32)
    for s in range(NCORES):
        o = results[s]["outp"]  # [128, 32, B, TS]
        for p in range(2):
            out[:, :, s * TS:(s + 1) * TS, 32 * p:32 * p + 32] = \
                np.moveaxis(o[64 * p:64 * p + 64], (0, 1, 2, 3), (1, 3, 0, 2))
    return out


def kernel(x, Wq, bq, aq, gq, betaq, Wk, bk, ak, gk, betak,
           Wv, bv, av, gv, betav, Wp, bp, ap, gp, betap):
    x = np.asarray(x, np.float32)
    for g_arr, be_arr in ((gq, betaq), (gk, betak), (gv, betav), (gp, betap)):
        assert np.all(np.asarray(g_arr) == 1.0), "affine gain != 1 unsupported"
        assert np.all(np.asarray(be_arr) == 0.0), "affine shift != 0 unsupported"
    for a_arr in (aq, ak, av, np.asarray(ap)[None]):
        a_np = np.asarray(a_arr)
        assert np.all((a_np >= 0) & (a_np <= 1)), "prelu alpha out of [0,1]"

    in_maps = make_inputs(x, np.asarray(Wq), np.asarray(bq), np.asarray(aq),
                          np.asarray(Wk), np.asarray(bk), np.asarray(ak),
                          np.asarray(Wv), np.asarray(bv), np.asarray(av),
                          np.asarray(Wp), np.asarray(bp), float(np.asarray(ap)))
    nc = build_kernel([list(range(NCORES))])
    res = run_bass_kernel_spmd(nc, in_maps, core_ids=list(range(NCORES)))
    return assemble_output(res.results)

